# revision 22
# baseline (speedup 1.0000x reference)
"""Trainium2 Bass kernel for nn_MultiLinearCentroids (vq_codebook).

Reference math per class c (C=100, F=128, E=2048, B=512):
  one spectral-norm power-iteration step:
    sigma_c = || W_c (W_c^T u_c) || / || W_c^T u_c ||
  z = x @ W_c^T / sigma_c + b_c                         [B, F]
  probs[:, c] = exp(-||c_c - z||^2 / 2)                 [B]

Sharding: class dim padded 100 -> 104 = 8 cores x 13 classes. x replicated.
Host does only layout transforms (transpose / slice / concat / dtype cast);
all math (including sigma) runs on device.

Key design points (vs. the fp32 LOW_HIGH baseline at 287us):
  - W, x, u ship as FP16: the PE runs fp16 matmuls at 1 cyc/row (4x the
    fp32 pair), weight loads get FWL, and HBM traffic halves to ~10MB --
    critical because all 8 cores share device HBM and the full-fp32
    working set trips the DMA utilization throttle. fp16's 2^-11 mantissa
    matches the tf32-style rounding fp32r would give anyway (~4e-3 rel
    err vs the 2e-2 gate).
  - Host pre-permutes W/x to per-partition-contiguous layouts so each DMA
    is a plain 2D copy: cheap SP triggers, full-row HBM reads.
  - t = W^T u per class: one [128, 2048] tensor_tensor (W .* u_bcast) on
    GpSimd (DVE for the last classes), then one segmented tensor_reduce
    on DVE into [128, 16] fp32, converted to a sparse fp16 layout (even
    chunk k at col k, odd at col k+31).
  - r = W t via grouped junk-block matmuls: lhsT = [128, 33] slice of the
    sparse t (even/odd t columns land at out partitions 0/32, both
    32-aligned for reads); rhs = the two matching W chunks [128, 256].
    8 matmuls/class with ~free 33-col LDWEIGHTS instead of 16 N=1
    matmuls with full 128-col weight reloads.
  - dots r.r and r.u (== ||W^T u||^2) as 1-partition DVE STT reduces;
    pairwise gpsimd.partition_broadcast + a width-2 batched 1/sigma
    chain: 1/sigma = exp(0.5 ln(ru/rr)) + one Newton step (Ln/Exp/Square
    all live in the natural_log_exp_and_others ACT table set -> single
    table load).
  - sq = Square(zT * invs + (b - c)) one ScalarE op -> fp16; dist2 =
    ones^T @ sq (fp16 PE partition reduce); probs row = Exp(-0.5 dist2),
    DMA'd out per class.
  - Software pipeline (stages per iteration it): GEMM(it-2) ->
    r/dots(it-3) -> chain(pair) -> Square/dist2/probs(it-5) -> t-mult(it)
    -> t-reduce(it-1), with per-engine emission ordered so sigma-tail
    work precedes stall-prone waits.
"""

import numpy as np

import concourse.bass as bass
import concourse.tile as tile
from concourse import bacc


class _Bacc(bacc.Bacc):
    """Bacc whose ACT-table pass only sees natural_log_exp_and_others.

    The default pass picks the first table set containing each function
    (natural_log for Ln, exp_and_others for Exp), which alternates sets
    every class = many table loads x ~2.7us. Ln, Exp and Square all live in
    natural_log_exp_and_others, so one load covers the whole kernel."""

    def insert_act_table_loads(self):
        from concourse.hw_specs import get_activation_tables
        has_activation = any(
            isinstance(i, bacc.mybir.InstActivation)
            for b in self.main_func.blocks
            for i in b.instructions
        )
        if not has_activation:
            return
        tables = [(k, v if k == "natural_log_exp_and_others" else type(v)())
                  for k, v in get_activation_tables(self.m.arch).items()]
        bacc._bass_rust.insert_act_table_loads(self, tables)


from concourse import mybir

B = 512
C = 100
E = 2048
F = 128
NCORES = 8
CPAD = 104
CL = CPAD // NCORES  # 13 classes per core
KCH = E // 128       # 16 contraction chunks
XGRP = 4             # x chunks per staging DMA
KF = KCH * F

# classes whose t-multiply runs on DVE instead of GpSimd (tail relief)
DVE_TT_CLASSES = frozenset((9, 10, 11, 12))

_NC = None


def _emit(tc, d):
    nc = tc.nc
    f32 = mybir.dt.float32
    f16 = mybir.dt.float16
    mult = mybir.AluOpType.mult
    add = mybir.AluOpType.add
    AF = mybir.ActivationFunctionType
    AX = mybir.AxisListType

    import contextlib
    ctx = contextlib.ExitStack()
    with ctx:
        singles = ctx.enter_context(tc.tile_pool(name="singles", bufs=1))
        wtp = ctx.enter_context(tc.tile_pool(name="wtp", bufs=8))
        pp = ctx.enter_context(tc.tile_pool(name="pp", bufs=2))
        sqp = ctx.enter_context(tc.tile_pool(name="sqp", bufs=2))
        smp = ctx.enter_context(tc.tile_pool(name="smp", bufs=4))
        zps = ctx.enter_context(tc.tile_pool(name="zps", bufs=4, space="PSUM"))
        rps = ctx.enter_context(tc.tile_pool(name="rps", bufs=2, space="PSUM"))
        d2p = ctx.enter_context(tc.tile_pool(name="d2p", bufs=2, space="PSUM"))

        # --- input staging. Two independent DGE queues: bulk W/x triggers
        # on SP, small/broadcast inputs on the ScalarE queue.
        ub = d["ubflat"]
        ubc_sb = singles.tile([128, CL * F], f16, tag="ubc")
        ub_b = bass.AP(tensor=ub.tensor, offset=ub.offset,
                       ap=[[0, 128]] + [list(a) for a in ub.ap])
        nc.scalar.dma_start(out=ubc_sb, in_=ub_b)
        # u broadcast materialized with the chunk repeat baked in, so the
        # t-multiply reads a contiguous [128, KCH*F] operand (the stride-0
        # repeat AP halves DVE/Pool tensor_tensor throughput)
        ubc2 = singles.tile([128, CL, KCH, F], f16, tag="ubc2")
        ur = d["ubrep"]
        ub2 = bass.AP(tensor=ur.tensor, offset=ur.offset,
                      ap=[[0, 128]] + [list(a) for a in ur.ap])
        nc.scalar.dma_start(
            out=ubc2.rearrange("p c k f -> p (c k f)"), in_=ub2)
        misc_sb = singles.tile([128, 2 * CL], f32, tag="misc")
        nc.scalar.dma_start(out=misc_sb, in_=d["misc"])
        m16_sb = singles.tile([128, 1], f16, tag="m16")
        nc.scalar.dma_start(out=m16_sb, in_=d["m16"][:, 0:1])
        t16 = singles.tile([128, 3, 48], f16, tag="t16")
        nc.scalar.dma_start(out=t16, in_=d["m16"][:, 1:145].rearrange(
            "p (a c) -> p a c", a=3))
        ones_sb = m16_sb[:, 0:1]

        # W trigger groups: two singles first (fast pipeline start), then
        # pairs; all issued in the prologue interleaved with x groups.
        WGROUPS = [[0], [1], [2, 3], [4, 5], [6, 7], [8, 9], [10, 11], [12]]
        wt_of = {}

        def wt_dma(gi):
            cls = WGROUPS[gi]
            wt = wtp.tile([128, len(cls), KCH, F], f16, tag="wt",
                          name=f"wtg{gi}")
            nc.sync.dma_start(
                out=wt, in_=d["wt"][:, cls[0] * KF:(cls[-1] + 1) * KF
                                    ].rearrange("p (c k f) -> p c k f",
                                                k=KCH, f=F))
            for i, c in enumerate(cls):
                wt_of[c] = (wt, i)

        def wtc(c):
            t, i = wt_of[c]
            return t[:, i, :, :]

        xt_tiles = []
        for g in range(KCH // XGRP):
            xg = singles.tile([128, XGRP, B], f16, tag=f"xt{g}",
                              name=f"xt{g}")
            nc.sync.dma_start(
                out=xg, in_=d["xt"][:, g * XGRP * B:(g + 1) * XGRP * B
                                    ].rearrange("p (k b) -> p k b", b=B))
            xt_tiles.append(xg)
            if g == 0:
                wt_dma(0)
                wt_dma(1)
            elif g < 3:
                wt_dma(g + 1)
        for gi in range(4, len(WGROUPS)):
            wt_dma(gi)

        negm_sb = singles.tile([F, CL], f32, tag="negm")
        nc.vector.tensor_sub(negm_sb, misc_sb[:, :CL], misc_sb[:, CL:])

        def xchunk(k):
            return xt_tiles[k // XGRP][:, k % XGRP, :]

        st = [dict() for _ in range(CL)]

        for it in range(CL + 5):
            # ------ B(it-2): main GEMM
            cb = it - 2
            if 0 <= cb < CL:
                s = st[cb]
                zT = zps.tile([F, B], f32, tag="zT")
                s["zT"] = zT
                wt = wtc(cb)
                for k in range(KCH):
                    nc.tensor.matmul(
                        zT, lhsT=wt[:, k, :], rhs=xchunk(k),
                        start=(k == 0), stop=(k == KCH - 1))

            # ------ B2(it-3): r via junk-block matmuls; dots
            cr = it - 3
            if 0 <= cr < CL:
                s = st[cr]
                wt, t_sb = wtc(cr), s["t"]
                rP = rps.tile([33, 256], f32, tag="rP")
                for j in range(KCH // 2):
                    nc.tensor.matmul(
                        rP, lhsT=t_sb[:, 2 * j:2 * j + 33],
                        rhs=wt[:, 2 * j:2 * j + 2, :].rearrange(
                            "p a f -> p (a f)"),
                        start=(j == 0), stop=(j == KCH // 2 - 1))
                rodd = smp.tile([1, F], f32, tag="rodd")
                nc.scalar.activation(out=rodd, in_=rP[32:33, 128:256],
                                     func=AF.Copy)
                rd = smp.tile([1, F], f32, tag="rd")
                nc.vector.tensor_add(rd, rP[0:1, 0:128], rodd)
                lane = cr % 2
                if lane == 0:
                    dots = smp.tile([1, 4], f32, tag="dots", name="dots")
                    s["dots"] = dots
                else:
                    dots = st[cr - 1]["dots"]
                    s["dots"] = dots
                scr1 = smp.tile([1, F], f32, tag="scr1")
                nc.vector.scalar_tensor_tensor(
                    out=scr1, in0=rd, scalar=1.0, in1=rd,
                    op0=mult, op1=mult,
                    accum_out=dots[0:1, 2 * lane:2 * lane + 1])
                nc.vector.scalar_tensor_tensor(
                    out=scr1, in0=rd, scalar=1.0,
                    in1=ubc_sb[0:1, cr * F:(cr + 1) * F],
                    op0=mult, op1=mult,
                    accum_out=dots[0:1, 2 * lane + 1:2 * lane + 2])

            # ------ C: per-pair broadcast + batched 1/sigma chain
            if 0 <= cr < CL and (cr % 2 == 1 or cr == CL - 1):
                w = 1 if (cr == CL - 1 and CL % 2 == 1) else 2
                dots = st[cr]["dots"]
                bc = smp.tile([128, 4], f32, tag="bc")
                nc.gpsimd.partition_broadcast(bc[:, 0:2 * w],
                                              dots[0:1, 0:2 * w])
                rrb = bass.AP(tensor=bc.tensor, offset=bc.offset,
                              ap=[list(bc.ap[0]), [2, w]])
                rub = bass.AP(tensor=bc.tensor, offset=bc.offset + 1,
                              ap=[list(bc.ap[0]), [2, w]])
                recip = smp.tile([128, 2], f32, tag="recip")
                nc.vector.reciprocal(recip[:, 0:w], rrb)
                invs2 = smp.tile([128, 2], f32, tag="invs2")
                nc.vector.tensor_mul(invs2[:, 0:w], recip[:, 0:w], rub)
                lnr = smp.tile([128, 2], f32, tag="lnr")
                nc.scalar.activation(out=lnr[:, 0:w], in_=invs2[:, 0:w],
                                     func=AF.Ln)
                invs0 = smp.tile([128, 2], f32, tag="invs0")
                nc.scalar.activation(out=invs0[:, 0:w], in_=lnr[:, 0:w],
                                     func=AF.Exp, scale=0.5)
                # one Newton step y1 = (y0 + a/y0)/2 tightens the LUT
                # exp(0.5 ln a) sqrt estimate to ~1 ulp; probs error is
                # ~600x the relative sigma error, so this matters.
                ry = smp.tile([128, 2], f32, tag="ry")
                nc.vector.reciprocal(ry[:, 0:w], invs0[:, 0:w])
                ar = smp.tile([128, 2], f32, tag="ar")
                nc.vector.tensor_mul(ar[:, 0:w], invs2[:, 0:w], ry[:, 0:w])
                hsum = smp.tile([128, 2], f32, tag="hsum")
                nc.vector.tensor_add(hsum[:, 0:w], invs0[:, 0:w],
                                     ar[:, 0:w])
                invs = smp.tile([128, 2], f32, tag="invs")
                nc.vector.tensor_scalar_mul(invs[:, 0:w], hsum[:, 0:w], 0.5)
                st[cr - w + 1]["invs"] = invs[:, 0:1]
                if w == 2:
                    st[cr]["invs"] = invs[:, 1:2]

            # ------ D(it-5): Square + dist2 + probs
            cd = it - 5
            if 0 <= cd:
                s = st[cd]
                sq = sqp.tile([F, B], f16, tag="sq")
                nc.scalar.activation(
                    out=sq, in_=s["zT"], func=AF.Square,
                    bias=negm_sb[:, cd:cd + 1], scale=s["invs"])
                d2 = d2p.tile([1, B], f32, tag="d2")
                nc.tensor.matmul(d2, lhsT=ones_sb, rhs=sq,
                                 start=True, stop=True)
                probs_c = smp.tile([1, B], f32, tag="probs_c")
                nc.scalar.activation(
                    out=probs_c, in_=d2, func=AF.Exp, scale=-0.5)
                nc.sync.dma_start(out=d["out"][0:1, cd * B:(cd + 1) * B],
                                  in_=probs_c)

            # ------ A(it): prod = W .* u_bc (Pool; last classes on DVE)
            if it < CL:
                s = st[it]
                t_sb = t16[:, it % 3, :]
                s["t"] = t_sb
                prod = pp.tile([128, KCH, F], f16, tag="prod")
                s["prod"] = prod
                teng = nc.vector if it in DVE_TT_CLASSES else nc.gpsimd
                teng.tensor_tensor(
                    out=prod, in0=wtc(it), in1=ubc2[:, it, :, :], op=mult)

            # ------ A2(it-1): t = segmented reduce (fp32) + fp16 scatter
            ca = it - 1
            if 0 <= ca < CL:
                s = st[ca]
                prod, t_sb = s["prod"], s["t"]
                t32 = smp.tile([128, 16], f32, tag="t32")
                pin = bass.AP(tensor=prod.tensor, offset=prod.offset,
                              ap=[list(prod.ap[0]), [F, 2], [2 * F, 8],
                                  [1, F]])
                nc.vector.tensor_reduce(
                    out=t32.rearrange("p (a c) -> p a c", a=2), in_=pin,
                    axis=AX.X, op=add)
                tout = bass.AP(tensor=t_sb.tensor, offset=t_sb.offset,
                               ap=[list(t_sb.ap[0]), [32, 2], [2, 8]])
                nc.vector.tensor_copy(
                    tout, t32.rearrange("p (a c) -> p a c", a=2))


def _build():
    nc = _Bacc(trn_type="TRN2", target_bir_lowering=False, debug=False,
               num_devices=NCORES)
    f32 = mybir.dt.float32
    f16 = mybir.dt.float16
    d = {
        "wt": nc.dram_tensor("wt", [128, CL * KCH * F], f16,
                             kind="ExternalInput").ap(),
        "xt": nc.dram_tensor("xt", [128, KCH * B], f16,
                             kind="ExternalInput").ap(),
        "misc": nc.dram_tensor("misc", [128, 2 * CL], f32,
                               kind="ExternalInput").ap(),
        "m16": nc.dram_tensor("m16", [128, 145], f16,
                              kind="ExternalInput").ap(),
        "ubflat": nc.dram_tensor("ubflat", [CL * F], f16,
                                 kind="ExternalInput").ap(),
        "ubrep": nc.dram_tensor("ubrep", [CL * KCH * F], f16,
                                kind="ExternalInput").ap(),
        "out": nc.dram_tensor("out", [1, CL * B], f32,
                              kind="ExternalOutput").ap(),
    }
    with tile.TileContext(nc) as tc:
        _emit(tc, d)
    nc.compile()
    return nc


def _get_nc():
    global _NC
    if _NC is None:
        _NC = _build()
    return _NC


def make_in_maps(inputs):
    x = np.ascontiguousarray(inputs["x"], dtype=np.float32)
    W = np.ascontiguousarray(inputs["W"], dtype=np.float32)
    b = np.ascontiguousarray(inputs["b"], dtype=np.float32)
    u = np.ascontiguousarray(inputs["u"], dtype=np.float32)
    c = np.ascontiguousarray(inputs["c"], dtype=np.float32)
    pad = CPAD - C
    Wp = np.concatenate([W, W[:pad]], axis=0)
    bp = np.concatenate([b, b[:pad]], axis=0)
    up = np.concatenate([u, u[:pad]], axis=0)
    cp = np.concatenate([c, c[:pad]], axis=0)
    # pre-permute to per-partition-contiguous fp16 layouts so device DMAs
    # are simple 2D copies (cheap SP triggers, full-row HBM reads):
    # wt[p, c, k, f] = W[c, f, 128k+p];  xt[p, k, b] = x[b, 128k+p]
    WT = Wp.transpose(0, 2, 1).reshape(CPAD, KCH, 128, F)
    xt = np.ascontiguousarray(x.T.reshape(KCH, 128, B).transpose(1, 0, 2)
                              .reshape(128, KCH * B).astype(np.float16))
    m16 = np.zeros((128, 145), dtype=np.float16)
    m16[:, 0] = 1.0
    in_maps = []
    for ci in range(NCORES):
        sl = slice(ci * CL, (ci + 1) * CL)
        in_maps.append({
            "wt": np.ascontiguousarray(
                WT[sl].transpose(2, 0, 1, 3).reshape(128, CL * KCH * F)
                .astype(np.float16)),
            "xt": xt,
            "ubflat": np.ascontiguousarray(
                up[sl].reshape(-1).astype(np.float16)),
            "ubrep": np.ascontiguousarray(
                np.tile(up[sl].astype(np.float16)[:, None, :],
                        (1, KCH, 1)).reshape(-1)),
            "misc": np.ascontiguousarray(
                np.concatenate([bp[sl].T, cp[sl].T], axis=1)),
            "m16": m16,
        })
    return in_maps


def run_spmd(in_maps, **kw):
    from concourse.bass_utils import run_bass_kernel_spmd
    return run_bass_kernel_spmd(_get_nc(), in_maps, list(range(NCORES)), **kw)


def gather_output(results):
    rows = np.concatenate(
        [results[i]["out"].reshape(CL, B) for i in range(NCORES)], axis=0)
    return np.ascontiguousarray(rows[:C].T)  # [B, C] float32


def kernel(**inputs):
    bkr = run_spmd(make_in_maps(inputs))
    return gather_output(bkr.results)


# revision 23
# speedup vs baseline: 1.1092x; 1.1092x over previous
"""Trainium2 Bass kernel for nn_MultiLinearCentroids (vq_codebook).

Reference math per class c (C=100, F=128, E=2048, B=512):
  one spectral-norm power-iteration step:
    sigma_c = || W_c (W_c^T u_c) || / || W_c^T u_c ||
  z = x @ W_c^T / sigma_c + b_c                         [B, F]
  probs[:, c] = exp(-||c_c - z||^2 / 2)                 [B]

Sharding: class dim padded 100 -> 104 = 8 cores x 13 classes. x replicated.
Host does only layout transforms (transpose / slice / concat / dtype cast);
all math (including sigma) runs on device.

Key design points (vs. the fp32 LOW_HIGH baseline at 287us):
  - W, x, u ship as FP16: the PE runs fp16 matmuls at 1 cyc/row (4x the
    fp32 pair), weight loads get FWL, and HBM traffic halves to ~10MB --
    critical because all 8 cores share device HBM and the full-fp32
    working set trips the DMA utilization throttle. fp16's 2^-11 mantissa
    matches the tf32-style rounding fp32r would give anyway (~4e-3 rel
    err vs the 2e-2 gate).
  - Host pre-permutes W/x to per-partition-contiguous layouts so each DMA
    is a plain 2D copy: cheap SP triggers, full-row HBM reads.
  - t = W^T u per class: one [128, 2048] tensor_tensor (W .* u_bcast) on
    GpSimd (DVE for the last classes), then one segmented tensor_reduce
    on DVE into [128, 16] fp32, converted to a sparse fp16 layout (even
    chunk k at col k, odd at col k+31).
  - r = W t via grouped junk-block matmuls: lhsT = [128, 33] slice of the
    sparse t (even/odd t columns land at out partitions 0/32, both
    32-aligned for reads); rhs = the two matching W chunks [128, 256].
    8 matmuls/class with ~free 33-col LDWEIGHTS instead of 16 N=1
    matmuls with full 128-col weight reloads.
  - dots r.r and r.u (== ||W^T u||^2) as 1-partition DVE STT reduces;
    pairwise gpsimd.partition_broadcast + a width-2 batched 1/sigma
    chain: 1/sigma = exp(0.5 ln(ru/rr)) + one Newton step (Ln/Exp/Square
    all live in the natural_log_exp_and_others ACT table set -> single
    table load).
  - sq = Square(zT * invs + (b - c)) one ScalarE op -> fp16; dist2 =
    ones^T @ sq (fp16 PE partition reduce); probs row = Exp(-0.5 dist2),
    DMA'd out per class.
  - Software pipeline (stages per iteration it): GEMM(it-2) ->
    r/dots(it-3) -> chain(pair) -> Square/dist2/probs(it-5) -> t-mult(it)
    -> t-reduce(it-1), with per-engine emission ordered so sigma-tail
    work precedes stall-prone waits.
"""

import numpy as np

import concourse.bass as bass
import concourse.tile as tile
from concourse import bacc


class _Bacc(bacc.Bacc):
    """Bacc whose ACT-table pass only sees natural_log_exp_and_others.

    The default pass picks the first table set containing each function
    (natural_log for Ln, exp_and_others for Exp), which alternates sets
    every class = many table loads x ~2.7us. Ln, Exp and Square all live in
    natural_log_exp_and_others, so one load covers the whole kernel."""

    def insert_act_table_loads(self):
        from concourse.hw_specs import get_activation_tables
        has_activation = any(
            isinstance(i, bacc.mybir.InstActivation)
            for b in self.main_func.blocks
            for i in b.instructions
        )
        if not has_activation:
            return
        tables = [(k, v if k == "natural_log_exp_and_others" else type(v)())
                  for k, v in get_activation_tables(self.m.arch).items()]
        bacc._bass_rust.insert_act_table_loads(self, tables)


from concourse import mybir

B = 512
C = 100
E = 2048
F = 128
NCORES = 8
CPAD = 104
CL = CPAD // NCORES  # 13 classes per core
KCH = E // 128       # 16 contraction chunks
XGRP = 4             # x chunks per staging DMA
KF = KCH * F

# classes whose t-multiply runs on DVE instead of GpSimd (tail relief)
DVE_TT_CLASSES = frozenset((10, 11, 12))

_NC = None


def _emit(tc, d):
    nc = tc.nc
    f32 = mybir.dt.float32
    f16 = mybir.dt.float16
    mult = mybir.AluOpType.mult
    add = mybir.AluOpType.add
    AF = mybir.ActivationFunctionType
    AX = mybir.AxisListType

    import contextlib
    ctx = contextlib.ExitStack()
    with ctx:
        singles = ctx.enter_context(tc.tile_pool(name="singles", bufs=1))
        wtp = ctx.enter_context(tc.tile_pool(name="wtp", bufs=8))
        pp = ctx.enter_context(tc.tile_pool(name="pp", bufs=2))
        sqp = ctx.enter_context(tc.tile_pool(name="sqp", bufs=2))
        smp = ctx.enter_context(tc.tile_pool(name="smp", bufs=4))
        zps = ctx.enter_context(tc.tile_pool(name="zps", bufs=4, space="PSUM"))
        rps = ctx.enter_context(tc.tile_pool(name="rps", bufs=2, space="PSUM"))
        d2p = ctx.enter_context(tc.tile_pool(name="d2p", bufs=2, space="PSUM"))

        # --- input staging. Two independent DGE queues: bulk W/x triggers
        # on SP, small/broadcast inputs on the ScalarE queue.
        ub = d["ubflat"]
        ubc_sb = singles.tile([128, CL * F], f16, tag="ubc")
        ub_b = bass.AP(tensor=ub.tensor, offset=ub.offset,
                       ap=[[0, 128]] + [list(a) for a in ub.ap])
        nc.scalar.dma_start(out=ubc_sb, in_=ub_b)
        # u broadcast with the chunk repeat baked in for the DVE-TT
        # classes: a contiguous fp16 in1 unlocks DVE's 2x 16-bit mode
        # (1.2us vs 4us with a stride-0 repeat AP). Pool classes keep the
        # stride-0 read -- GpSimd runs ~4us either way.
        ubc2 = singles.tile([128, len(DVE_TT_CLASSES), KCH, F], f16,
                            tag="ubc2")
        ur = d["ubrep"]
        ub2 = bass.AP(tensor=ur.tensor, offset=ur.offset,
                      ap=[[0, 128]] + [list(a) for a in ur.ap])
        nc.scalar.dma_start(
            out=ubc2.rearrange("p c k f -> p (c k f)"), in_=ub2)
        misc_sb = singles.tile([128, 2 * CL], f32, tag="misc")
        nc.scalar.dma_start(out=misc_sb, in_=d["misc"])
        m16_sb = singles.tile([128, 1], f16, tag="m16")
        nc.scalar.dma_start(out=m16_sb, in_=d["m16"][:, 0:1])
        t16 = singles.tile([128, 3, 48], f16, tag="t16")
        nc.scalar.dma_start(out=t16, in_=d["m16"][:, 1:145].rearrange(
            "p (a c) -> p a c", a=3))
        ones_sb = m16_sb[:, 0:1]

        # W trigger groups: two singles first (fast pipeline start), then
        # pairs; all issued in the prologue interleaved with x groups.
        WGROUPS = [[0], [1], [2, 3], [4, 5], [6, 7], [8, 9], [10, 11], [12]]
        wt_of = {}

        def wt_dma(gi):
            cls = WGROUPS[gi]
            wt = wtp.tile([128, len(cls), KCH, F], f16, tag="wt",
                          name=f"wtg{gi}")
            nc.sync.dma_start(
                out=wt, in_=d["wt"][:, cls[0] * KF:(cls[-1] + 1) * KF
                                    ].rearrange("p (c k f) -> p c k f",
                                                k=KCH, f=F))
            for i, c in enumerate(cls):
                wt_of[c] = (wt, i)

        def wtc(c):
            t, i = wt_of[c]
            return t[:, i, :, :]

        xt_tiles = []
        for g in range(KCH // XGRP):
            xg = singles.tile([128, XGRP, B], f16, tag=f"xt{g}",
                              name=f"xt{g}")
            nc.sync.dma_start(
                out=xg, in_=d["xt"][:, g * XGRP * B:(g + 1) * XGRP * B
                                    ].rearrange("p (k b) -> p k b", b=B))
            xt_tiles.append(xg)
            if g == 0:
                wt_dma(0)
                wt_dma(1)
            elif g < 3:
                wt_dma(g + 1)
        for gi in range(4, len(WGROUPS)):
            wt_dma(gi)

        negm_sb = singles.tile([F, CL], f32, tag="negm")
        nc.vector.tensor_sub(negm_sb, misc_sb[:, :CL], misc_sb[:, CL:])

        def xchunk(k):
            return xt_tiles[k // XGRP][:, k % XGRP, :]

        st = [dict() for _ in range(CL)]

        for it in range(CL + 5):
            # ------ B(it-2): main GEMM
            cb = it - 2
            if 0 <= cb < CL:
                s = st[cb]
                zT = zps.tile([F, B], f32, tag="zT")
                s["zT"] = zT
                wt = wtc(cb)
                for k in range(KCH):
                    nc.tensor.matmul(
                        zT, lhsT=wt[:, k, :], rhs=xchunk(k),
                        start=(k == 0), stop=(k == KCH - 1))

            # ------ B2(it-3): r via junk-block matmuls; dots
            cr = it - 3
            if 0 <= cr < CL:
                s = st[cr]
                wt, t_sb = wtc(cr), s["t"]
                rP = rps.tile([33, 256], f32, tag="rP")
                for j in range(KCH // 2):
                    nc.tensor.matmul(
                        rP, lhsT=t_sb[:, 2 * j:2 * j + 33],
                        rhs=wt[:, 2 * j:2 * j + 2, :].rearrange(
                            "p a f -> p (a f)"),
                        start=(j == 0), stop=(j == KCH // 2 - 1))
                rodd = smp.tile([1, F], f32, tag="rodd")
                nc.scalar.activation(out=rodd, in_=rP[32:33, 128:256],
                                     func=AF.Copy)
                rd = smp.tile([1, F], f32, tag="rd")
                nc.vector.tensor_add(rd, rP[0:1, 0:128], rodd)
                lane = cr % 2
                if lane == 0:
                    dots = smp.tile([1, 4], f32, tag="dots", name="dots")
                    s["dots"] = dots
                else:
                    dots = st[cr - 1]["dots"]
                    s["dots"] = dots
                scr1 = smp.tile([1, F], f32, tag="scr1")
                nc.vector.scalar_tensor_tensor(
                    out=scr1, in0=rd, scalar=1.0, in1=rd,
                    op0=mult, op1=mult,
                    accum_out=dots[0:1, 2 * lane:2 * lane + 1])
                nc.vector.scalar_tensor_tensor(
                    out=scr1, in0=rd, scalar=1.0,
                    in1=ubc_sb[0:1, cr * F:(cr + 1) * F],
                    op0=mult, op1=mult,
                    accum_out=dots[0:1, 2 * lane + 1:2 * lane + 2])

            # ------ C: per-pair broadcast + batched 1/sigma chain
            if 0 <= cr < CL and (cr % 2 == 1 or cr == CL - 1):
                w = 1 if (cr == CL - 1 and CL % 2 == 1) else 2
                dots = st[cr]["dots"]
                bc = smp.tile([128, 4], f32, tag="bc")
                nc.gpsimd.partition_broadcast(bc[:, 0:2 * w],
                                              dots[0:1, 0:2 * w])
                rrb = bass.AP(tensor=bc.tensor, offset=bc.offset,
                              ap=[list(bc.ap[0]), [2, w]])
                rub = bass.AP(tensor=bc.tensor, offset=bc.offset + 1,
                              ap=[list(bc.ap[0]), [2, w]])
                recip = smp.tile([128, 2], f32, tag="recip")
                nc.vector.reciprocal(recip[:, 0:w], rrb)
                invs2 = smp.tile([128, 2], f32, tag="invs2")
                nc.vector.tensor_mul(invs2[:, 0:w], recip[:, 0:w], rub)
                lnr = smp.tile([128, 2], f32, tag="lnr")
                nc.scalar.activation(out=lnr[:, 0:w], in_=invs2[:, 0:w],
                                     func=AF.Ln)
                invs0 = smp.tile([128, 2], f32, tag="invs0")
                nc.scalar.activation(out=invs0[:, 0:w], in_=lnr[:, 0:w],
                                     func=AF.Exp, scale=0.5)
                # one Newton step y1 = (y0 + a/y0)/2 tightens the LUT
                # exp(0.5 ln a) sqrt estimate to ~1 ulp; probs error is
                # ~600x the relative sigma error, so this matters.
                ry = smp.tile([128, 2], f32, tag="ry")
                nc.vector.reciprocal(ry[:, 0:w], invs0[:, 0:w])
                ar = smp.tile([128, 2], f32, tag="ar")
                nc.vector.tensor_mul(ar[:, 0:w], invs2[:, 0:w], ry[:, 0:w])
                hsum = smp.tile([128, 2], f32, tag="hsum")
                nc.vector.tensor_add(hsum[:, 0:w], invs0[:, 0:w],
                                     ar[:, 0:w])
                invs = smp.tile([128, 2], f32, tag="invs")
                nc.vector.tensor_scalar_mul(invs[:, 0:w], hsum[:, 0:w], 0.5)
                st[cr - w + 1]["invs"] = invs[:, 0:1]
                if w == 2:
                    st[cr]["invs"] = invs[:, 1:2]

            # ------ D(it-5): Square + dist2 + probs
            cd = it - 5
            if 0 <= cd:
                s = st[cd]
                sq = sqp.tile([F, B], f16, tag="sq")
                nc.scalar.activation(
                    out=sq, in_=s["zT"], func=AF.Square,
                    bias=negm_sb[:, cd:cd + 1], scale=s["invs"])
                d2 = d2p.tile([1, B], f32, tag="d2")
                nc.tensor.matmul(d2, lhsT=ones_sb, rhs=sq,
                                 start=True, stop=True)
                probs_c = smp.tile([1, B], f32, tag="probs_c")
                nc.scalar.activation(
                    out=probs_c, in_=d2, func=AF.Exp, scale=-0.5)
                nc.sync.dma_start(out=d["out"][0:1, cd * B:(cd + 1) * B],
                                  in_=probs_c)

            # ------ A(it): prod = W .* u_bc (Pool; last classes on DVE)
            if it < CL:
                s = st[it]
                t_sb = t16[:, it % 3, :]
                s["t"] = t_sb
                prod = pp.tile([128, KCH, F], f16, tag="prod")
                s["prod"] = prod
                if it in DVE_TT_CLASSES:
                    nc.vector.tensor_tensor(
                        out=prod, in0=wtc(it),
                        in1=ubc2[:, it - (CL - len(DVE_TT_CLASSES)), :, :],
                        op=mult)
                else:
                    uslice = ubc_sb[:, it * F:(it + 1) * F]
                    urep = bass.AP(
                        tensor=uslice.tensor, offset=uslice.offset,
                        ap=[list(uslice.ap[0]), [0, KCH]]
                        + [list(a) for a in uslice.ap[1:]])
                    nc.gpsimd.tensor_tensor(
                        out=prod, in0=wtc(it), in1=urep, op=mult)

            # ------ A2(it-1): t = segmented reduce (fp32) + fp16 scatter
            ca = it - 1
            if 0 <= ca < CL:
                s = st[ca]
                prod, t_sb = s["prod"], s["t"]
                t32 = smp.tile([128, 16], f32, tag="t32")
                nc.vector.tensor_reduce(
                    out=t32, in_=prod, axis=AX.X, op=add)
                tout = bass.AP(tensor=t_sb.tensor, offset=t_sb.offset,
                               ap=[list(t_sb.ap[0]), [2, 8], [32, 2]])
                nc.vector.tensor_copy(
                    tout, t32.rearrange("p (j b) -> p j b", b=2))


def _build():
    nc = _Bacc(trn_type="TRN2", target_bir_lowering=False, debug=False,
               num_devices=NCORES)
    f32 = mybir.dt.float32
    f16 = mybir.dt.float16
    d = {
        "wt": nc.dram_tensor("wt", [128, CL * KCH * F], f16,
                             kind="ExternalInput").ap(),
        "xt": nc.dram_tensor("xt", [128, KCH * B], f16,
                             kind="ExternalInput").ap(),
        "misc": nc.dram_tensor("misc", [128, 2 * CL], f32,
                               kind="ExternalInput").ap(),
        "m16": nc.dram_tensor("m16", [128, 145], f16,
                              kind="ExternalInput").ap(),
        "ubflat": nc.dram_tensor("ubflat", [CL * F], f16,
                                 kind="ExternalInput").ap(),
        "ubrep": nc.dram_tensor("ubrep", [3 * KCH * F], f16,
                                kind="ExternalInput").ap(),
        "out": nc.dram_tensor("out", [1, CL * B], f32,
                              kind="ExternalOutput").ap(),
    }
    with tile.TileContext(nc) as tc:
        _emit(tc, d)
    nc.compile()
    return nc


def _get_nc():
    global _NC
    if _NC is None:
        _NC = _build()
    return _NC


def make_in_maps(inputs):
    x = np.ascontiguousarray(inputs["x"], dtype=np.float32)
    W = np.ascontiguousarray(inputs["W"], dtype=np.float32)
    b = np.ascontiguousarray(inputs["b"], dtype=np.float32)
    u = np.ascontiguousarray(inputs["u"], dtype=np.float32)
    c = np.ascontiguousarray(inputs["c"], dtype=np.float32)
    pad = CPAD - C
    Wp = np.concatenate([W, W[:pad]], axis=0)
    bp = np.concatenate([b, b[:pad]], axis=0)
    up = np.concatenate([u, u[:pad]], axis=0)
    cp = np.concatenate([c, c[:pad]], axis=0)
    # pre-permute to per-partition-contiguous fp16 layouts so device DMAs
    # are simple 2D copies (cheap SP triggers, full-row HBM reads):
    # wt[p, c, k, f] = W[c, f, 128k+p];  xt[p, k, b] = x[b, 128k+p]
    WT = Wp.transpose(0, 2, 1).reshape(CPAD, KCH, 128, F)
    xt = np.ascontiguousarray(x.T.reshape(KCH, 128, B).transpose(1, 0, 2)
                              .reshape(128, KCH * B).astype(np.float16))
    m16 = np.zeros((128, 145), dtype=np.float16)
    m16[:, 0] = 1.0
    in_maps = []
    for ci in range(NCORES):
        sl = slice(ci * CL, (ci + 1) * CL)
        in_maps.append({
            "wt": np.ascontiguousarray(
                WT[sl].transpose(2, 0, 1, 3).reshape(128, CL * KCH * F)
                .astype(np.float16)),
            "xt": xt,
            "ubflat": np.ascontiguousarray(
                up[sl].reshape(-1).astype(np.float16)),
            "ubrep": np.ascontiguousarray(
                np.tile(up[sl][CL - 3:].astype(np.float16)[:, None, :],
                        (1, KCH, 1)).reshape(-1)),
            "misc": np.ascontiguousarray(
                np.concatenate([bp[sl].T, cp[sl].T], axis=1)),
            "m16": m16,
        })
    return in_maps


def run_spmd(in_maps, **kw):
    from concourse.bass_utils import run_bass_kernel_spmd
    return run_bass_kernel_spmd(_get_nc(), in_maps, list(range(NCORES)), **kw)


def gather_output(results):
    rows = np.concatenate(
        [results[i]["out"].reshape(CL, B) for i in range(NCORES)], axis=0)
    return np.ascontiguousarray(rows[:C].T)  # [B, C] float32


def kernel(**inputs):
    bkr = run_spmd(make_in_maps(inputs))
    return gather_output(bkr.results)
